# revision 6
# baseline (speedup 1.0000x reference)
"""Trainium2 Bass kernel v2 for nn_MultiHeadCovProbeV2.

Data-parallel over batch B=8: core i processes batch i.

Key differences vs v1 baseline:
  - x is cast to bf16 on host: halves HBM traffic (32MB -> 16MB/core).
  - x is ingested PRE-TRANSPOSED via the DMA xbar transpose engine
    (dma_start(transpose=True)), so the PE does no x-transposes and the
    DVE does no PSUM->SBUF x copies. PE only does the projection
    (bf16, 1 cyc/row), back-transposes of the small [128,S] projection,
    cov accumulation, and the Newton-Schulz tail.
  - bias-add + mask-mul fused into one DVE scalar_tensor_tensor.
  - weight/head tensors are pre-laid-out on host (transposed, stacked).
"""
import sys

for p in ("/opt/trn_rl_repo", "/root/.axon_site/_ro/trn_rl_repo"):
    if p not in sys.path:
        sys.path.append(p)

import numpy as np
import ml_dtypes
import concourse.bass as bass
import concourse.mybir as mybir
from concourse.tile import TileContext
from concourse.masks import make_identity

F32 = mybir.dt.float32
BF16 = mybir.dt.bfloat16
ALU = mybir.AluOpType

B = 8
S = 2048
D = 4096
H = 64          # d_hidden
DP = 128        # d_probe
HEADS = (10, 100, 1)
HTOT = sum(HEADS)
EPS = 1e-3
NITER = 3
N_CORES = 8

SB = 256                 # s-block width (proj moving dim / pipeline grain)
NSB = S // SB            # 8
TPB = SB // 128          # s-tiles per block = 2
DCH = D // 128           # 32 d chunks
NT = S // 128            # 16


def _split_multi_waits(nc):
    """This toolchain's walrus encodes at most one sem-wait per instruction —
    and none at all on InstDmaTransposeAnt (xbar transpose: hardware encodes
    only the completion sem_num; attached waits mis-encode and hang/crash the
    queue). Move excess waits onto single-wait NOPs preceding the
    instruction on the same engine sequencer, which preserves ordering
    (the sequencer blocks on the NOP before issuing the DMA trigger)."""
    n = 0
    for f in nc.m.functions:
        for bb in f.blocks:
            out = []
            changed = False
            for inst in bb.instructions:
                si = inst.sync_info
                limit = 0 if isinstance(inst, mybir.InstDmaTransposeAnt) \
                    else 1
                if si is not None and si.on_wait and len(si.on_wait) > limit:
                    waits = list(si.on_wait)
                    keep = waits[-limit:] if limit else []
                    move = waits[:-limit] if limit else waits
                    for w in move:
                        n += 1
                        nop = mybir.InstNoOp(name=f"I-wsplit-{n}",
                                             engine=inst.engine)
                        nop.sync_info = mybir.SyncInfo(on_wait=[w],
                                                       on_update=[])
                        out.append(nop)
                    si.on_wait = keep
                    changed = True
                out.append(inst)
            if changed:
                bb.instructions = out
    return n


def build_program(split=True, reps=1, stage=99, use_act_q=False,
                  skip=(), debug_out=None):
    nc = bass.Bass()
    x = nc.dram_tensor("x", [S, D], BF16, kind="ExternalInput")
    # host-prearranged: wT[p, k*128+m] = W[m, k*128+p] (chunk-transposed)
    wT = nc.dram_tensor("wT", [128, DCH * 128], BF16, kind="ExternalInput")
    bias_col_d = nc.dram_tensor("bias_col", [2 * H, 1], F32,
                                kind="ExternalInput")
    mask_bc_d = nc.dram_tensor("mask_bc", [128, S], BF16,
                               kind="ExternalInput")
    recip_d = nc.dram_tensor("recip_col", [H, 1], F32, kind="ExternalInput")
    hlT_d = nc.dram_tensor("hlT_all", [H, 3 * DP], F32, kind="ExternalInput")
    hr_d = nc.dram_tensor("hr_all", [H, 3 * DP], F32, kind="ExternalInput")
    woT_d = [nc.dram_tensor(f"woT{n}", [DP, hs], F32, kind="ExternalInput")
             for n, hs in enumerate(HEADS)]
    brow_d = nc.dram_tensor("bias_row", [1, HTOT], F32, kind="ExternalInput")
    out = nc.dram_tensor("out", [1, HTOT], F32, kind="ExternalOutput")
    dbg = nc.dram_tensor("dbg", [H, H], F32, kind="ExternalOutput") \
        if debug_out else None

    with TileContext(nc) as tc, \
         tc.tile_pool(name="const", bufs=1) as const, \
         tc.tile_pool(name="xT", bufs=NSB) as xT_pool, \
         tc.tile_pool(name="lr", bufs=2) as lr_pool, \
         tc.tile_pool(name="lrT", bufs=2) as lrT_pool, \
         tc.tile_pool(name="ns", bufs=1) as ns_pool, \
         tc.tile_pool(name="pj", bufs=2, space="PSUM") as pj_ps, \
         tc.tile_pool(name="bt", bufs=2, space="PSUM") as bt_ps, \
         tc.tile_pool(name="sm", bufs=2, space="PSUM") as sm_ps, \
         tc.tile_pool(name="cv", bufs=2, space="PSUM") as cov_ps:

        # ---- constants ----
        ident_f = const.tile([128, 128], F32)
        ident_b = const.tile([128, 128], BF16)
        ones_col = const.tile([128, 1], F32)
        ones_row = const.tile([1, 128], F32)
        i15 = const.tile([H, H], F32)
        epsI = const.tile([H, H], F32)
        if "ident" not in skip:
            make_identity(nc, ident_f[:, :])
            nc.vector.tensor_copy(ident_b[:, :], ident_f[:, :])
            nc.vector.memset(ones_col[:, :], 1.0)
            nc.vector.memset(ones_row[:, :], 1.0)
            nc.vector.tensor_scalar_mul(i15[:, :], ident_f[0:H, 0:H], 1.5)
            nc.vector.tensor_scalar_mul(epsI[:, :], ident_f[0:H, 0:H], EPS)
        if "act" not in skip:
            act_warm = const.tile([1, 1], F32)
            nc.scalar.activation(act_warm[:, :], ones_col[0:1, 0:1],
                                 mybir.ActivationFunctionType.Sqrt)

        # ---- small input loads (host-prepped layouts) ----
        wTs = const.tile([128, DCH * 128], BF16)      # chunk k at cols k*128
        if "wts" not in skip:
            nc.sync.dma_start(out=wTs[:, :], in_=wT[:, :])
        bias_col = const.tile([128, 1], F32)
        mask_bc = const.tile([128, S], BF16)
        recip_col = const.tile([H, 1], F32)
        if "mask" not in skip:
            nc.sync.dma_start(out=bias_col[:, :], in_=bias_col_d[:, :])
            nc.sync.dma_start(out=mask_bc[:, :], in_=mask_bc_d[:, :])
            nc.sync.dma_start(out=recip_col[:, :], in_=recip_d[:, :])
        hlT_all = const.tile([H, 3 * DP], F32)
        hr_all = const.tile([H, 3 * DP], F32)
        woT = []
        for n, hs in enumerate(HEADS):
            woT.append(const.tile([DP, hs], F32, name=f"woT{n}",
                                  tag=f"woT{n}"))
        if "heads" not in skip:
            nc.sync.dma_start(out=hlT_all[:, :], in_=hlT_d[:, :])
            nc.sync.dma_start(out=hr_all[:, :], in_=hr_d[:, :])
            for n in range(3):
                nc.sync.dma_start(out=woT[n][:, :], in_=woT_d[n][:, :])
        bias_row = const.tile([1, HTOT], F32)
        nc.sync.dma_start(out=bias_row[:, :], in_=brow_d[:, :])

        def bcast_col(name, val_ap, p=H):
            ps = sm_ps.tile([128, 384], F32, name=f"pb_{name}", tag="sm")
            nc.tensor.matmul(ps[0:p, 0:1], ones_row[0:1, 0:p], val_ap,
                             start=True, stop=True)
            col = const.tile([p, 1], F32, name=f"bc_{name}", tag=f"bc_{name}")
            nc.vector.tensor_copy(col[:, :], ps[0:p, 0:1])
            return col

        # ---- main pipeline ----
        for rep in range(reps):
            rs = f"r{rep}_" if reps > 1 else ""

            if stage < 1:
                out_sb0 = ns_pool.tile([1, HTOT], F32, name=f"{rs}o0",
                                       tag="osb")
                nc.vector.tensor_copy(out_sb0[:, :], bias_row[:, :])
                nc.sync.dma_start(out=out[:, :], in_=out_sb0[:, :])
                continue

            cov = cov_ps.tile([H, H], F32, name=f"{rs}cov", tag="cv")

            for blk in range(NSB):
                xTb = xT_pool.tile([128, DCH * SB], BF16,
                                   name=f"{rs}xT{blk}", tag="xT")
                if stage == -1:
                    # natural-ingest bandwidth probe: plain contiguous loads
                    nc.sync.dma_start(
                        out=xTb[:, :].rearrange("p (t c) -> t p c", t=TPB),
                        in_=x[blk * SB:(blk + 1) * SB, :].rearrange(
                            "(t p) c -> t p c", p=128))
                    continue
                # one xbar DMA per s-block: 3D dest [128, k, s], source rows
                # are full 8KB x rows (M2S >=4KB concat keeps DMA efficient)
                q = nc.sync if (not use_act_q or blk % 2 == 0) else nc.scalar
                q.dma_start(
                    out=xTb[:, :].rearrange("p (k s) -> p k s", k=DCH),
                    in_=x[blk * SB:(blk + 1) * SB, :],
                    transpose=True)
                if stage < 2:
                    continue

                pp = pj_ps.tile([128, SB], F32, name=f"{rs}pp{blk}", tag="pj")
                for k in range(DCH):
                    nc.tensor.matmul(pp[:, :], wTs[:, k * 128:(k + 1) * 128],
                                     xTb[:, k * SB:(k + 1) * SB],
                                     start=(k == 0), stop=(k == DCH - 1))
                # lr = (pp + bias) * mask  (bias per-partition, mask per-col)
                lr = lr_pool.tile([128, SB], BF16, name=f"{rs}lr{blk}",
                                  tag="lr")
                nc.vector.scalar_tensor_tensor(
                    lr[:, :], pp[:, :], bias_col[:, 0:1],
                    mask_bc[:, blk * SB:(blk + 1) * SB],
                    op0=ALU.add, op1=ALU.mult)
                if stage < 3:
                    continue

                btp = bt_ps.tile([128, SB], BF16, name=f"{rs}bt{blk}",
                                 tag="bt")
                for j in range(TPB):
                    nc.tensor.transpose(btp[:, j * 128:(j + 1) * 128],
                                        lr[:, j * 128:(j + 1) * 128],
                                        ident_b[:, :])
                lrT = lrT_pool.tile([128, SB], BF16, name=f"{rs}lrT{blk}",
                                    tag="lrT")
                nc.vector.tensor_copy(lrT[:, :], btp[:, :])
                for j in range(TPB):
                    t = blk * TPB + j
                    nc.tensor.matmul(cov[:, :],
                                     lrT[:, j * 128:j * 128 + H],
                                     lrT[:, j * 128 + H:(j + 1) * 128],
                                     start=(t == 0), stop=(t == NT - 1))

            if stage < 4:
                out_sb0 = ns_pool.tile([1, HTOT], F32, name=f"{rs}o0",
                                       tag="osb")
                if stage == 3:
                    nc.vector.tensor_copy(out_sb0[0:1, 0:H],
                                          cov[0:1, 0:H])
                    nc.vector.tensor_copy(out_sb0[0:1, H:HTOT],
                                          bias_row[0:1, H:HTOT])
                else:
                    nc.vector.tensor_copy(out_sb0[:, :], bias_row[:, :])
                nc.sync.dma_start(out=out[:, :], in_=out_sb0[:, :])
                continue

            # ---- A = cov*recipL + eps*I ----
            A = ns_pool.tile([H, H], F32, name=f"{rs}A", tag="A")
            nc.vector.scalar_tensor_tensor(A[:, :], cov[:, :],
                                           recip_col[:, 0:1], epsI[:, :],
                                           op0=ALU.mult, op1=ALU.add)

            if debug_out == "A":
                nc.sync.dma_start(out=dbg[:, :], in_=A[:, :])

            # ---- Frobenius norm ----
            scr = ns_pool.tile([H, H], F32, name=f"{rs}scr", tag="scr")
            sq_col = ns_pool.tile([H, 1], F32, name=f"{rs}sq", tag="sq")
            nc.vector.tensor_tensor(out=scr[:, :], in0=A[:, :], in1=A[:, :],
                                    op=ALU.mult)
            nc.vector.reduce_sum(sq_col[:, :], scr[:, :],
                                 axis=mybir.AxisListType.X)
            pS = sm_ps.tile([1, 384], F32, name=f"{rs}pS", tag="sm")
            nc.tensor.matmul(pS[0:1, 0:1], sq_col[:, :], ones_col[0:H, :],
                             start=True, stop=True)
            normA = ns_pool.tile([1, 1], F32, name=f"{rs}normA", tag="normA")
            nc.scalar.activation(normA[:, :], pS[0:1, 0:1],
                                 mybir.ActivationFunctionType.Sqrt)
            snorm = ns_pool.tile([1, 1], F32, name=f"{rs}sn", tag="sn")
            nc.scalar.activation(snorm[:, :], normA[:, :],
                                 mybir.ActivationFunctionType.Sqrt)
            rnorm = ns_pool.tile([1, 1], F32, name=f"{rs}rn", tag="rn")
            nc.vector.reciprocal(rnorm[:, :], normA[:, :])
            rnorm_h = bcast_col(f"{rs}rnorm", rnorm[0:1, 0:1])
            snorm_h = bcast_col(f"{rs}snorm", snorm[0:1, 0:1])

            # ---- Newton-Schulz, transpose-tracked, T1 = 1.5I - 0.5*Y0 ----
            def mm(name, lhsT, rhs, m=H, n=H):
                ps = sm_ps.tile([128, 384], F32, name=f"{rs}ps_{name}",
                                tag="sm")
                nc.tensor.matmul(ps[0:m, 0:n], lhsT, rhs, start=True,
                                 stop=True)
                return ps[0:m, 0:n]

            def to_sb(name, ps_ap, m=H, n=H):
                sb = ns_pool.tile([m, n], F32, name=f"{rs}{name}", tag=name)
                nc.vector.tensor_copy(sb[:, :], ps_ap)
                return sb

            def half_i15(name, src_ap):
                t = ns_pool.tile([H, H], F32, name=f"{rs}{name}", tag=name)
                nc.vector.scalar_tensor_tensor(t[:, :], src_ap, -0.5,
                                               i15[:, :], op0=ALU.mult,
                                               op1=ALU.add)
                return t

            Y0 = ns_pool.tile([H, H], F32, name=f"{rs}Y0", tag="Y0")
            nc.vector.tensor_scalar_mul(Y0[:, :], A[:, :], rnorm_h[:, 0:1])
            Yt0 = to_sb("Yt0", mm("yt0", Y0[:, :], ident_f[0:H, 0:H]))
            T1 = half_i15("T1", Y0[:, :])
            T1t = half_i15("T1t", Yt0[:, :])
            Y1 = to_sb("Y1", mm("y1", Yt0[:, :], T1[:, :]))
            Yt1 = to_sb("Yt1", mm("yt1", T1[:, :], Yt0[:, :]))
            W2 = mm("w2", T1t[:, :], Y1[:, :])
            W2t = mm("w2t", Y1[:, :], T1t[:, :])
            T2 = half_i15("T2", W2)
            T2t = half_i15("T2t", W2t)
            Y2 = to_sb("Y2", mm("y2", Yt1[:, :], T2[:, :]))
            Yt2 = to_sb("Yt2", mm("yt2", T2[:, :], Yt1[:, :]))
            Z2t = to_sb("Z2t", mm("z2t", T1[:, :], T2t[:, :]))
            W3 = mm("w3", Z2t[:, :], Y2[:, :])
            T3 = half_i15("T3", W3)
            Y3 = mm("y3", Yt2[:, :], T3[:, :])
            Ys = ns_pool.tile([H, H], F32, name=f"{rs}Ys", tag="Ys")
            nc.vector.tensor_scalar_mul(Ys[:, :], Y3, snorm_h[:, 0:1])
            if debug_out == "Ys":
                nc.sync.dma_start(out=dbg[:, :], in_=Ys[:, :])

            # ---- heads: hidden[n,h] = sum_{l,r} Ys[l,r] hl[n,h,l] hr[n,h,r]
            MhT = mm("mht", Ys[:, :], hlT_all[:, :], m=H, n=3 * DP)
            prod = ns_pool.tile([H, 3 * DP], F32, name=f"{rs}prod",
                                tag="prod")
            nc.vector.tensor_tensor(out=prod[:, :], in0=MhT,
                                    in1=hr_all[:, :], op=ALU.mult)
            hid_ps = sm_ps.tile([128, 384], F32, name=f"{rs}hid", tag="sm")
            for n in range(3):
                nc.tensor.matmul(hid_ps[:, n:n + 1],
                                 prod[:, n * DP:(n + 1) * DP],
                                 ones_col[0:H, 0:1], start=True, stop=True)
            hid_sb = ns_pool.tile([128, 3], F32, name=f"{rs}hsb", tag="hsb")
            nc.vector.tensor_copy(hid_sb[:, :], hid_ps[:, 0:3])
            out_ps = sm_ps.tile([1, 384], F32, name=f"{rs}ops", tag="sm")
            off = 0
            for n, hs in enumerate(HEADS):
                nc.tensor.matmul(out_ps[0:1, off:off + hs],
                                 hid_sb[:, n:n + 1], woT[n][:, :],
                                 start=True, stop=True)
                off += hs
            out_sb = ns_pool.tile([1, HTOT], F32, name=f"{rs}osb", tag="osb")
            nc.vector.tensor_tensor(out=out_sb[:, :], in0=out_ps[0:1, 0:HTOT],
                                    in1=bias_row[:, :], op=ALU.add)
            nc.sync.dma_start(out=out[:, :], in_=out_sb[:, :])

    if split:
        _split_multi_waits(nc)
    return nc


def make_in_maps(inputs):
    x = np.asarray(inputs["x"])
    mask = np.asarray(inputs["attn_mask"])
    xb = x.astype(ml_dtypes.bfloat16)

    wl = np.asarray(inputs["proj_left_w"], np.float32)
    wr = np.asarray(inputs["proj_right_w"], np.float32)
    w_st = np.concatenate([wl, wr], axis=0)              # [128, D]
    # wT[p, c*128+m] = W[m, c*128+p]: per-chunk transpose, laid out flat
    wT = np.ascontiguousarray(
        w_st.reshape(128, D // 128, 128).transpose(2, 1, 0).reshape(128, D)
    ).astype(ml_dtypes.bfloat16)
    bias_col = np.concatenate(
        [np.asarray(inputs["proj_left_b"], np.float32),
         np.asarray(inputs["proj_right_b"], np.float32)]).reshape(2 * H, 1)

    hlw = np.asarray(inputs["head_left"], np.float32)    # [3, DP, H]
    hrw = np.asarray(inputs["head_right"], np.float32)
    # hlT_all[l, n*DP+h] = hl[n, h, l];  hr_all[r, n*DP+h] = hr[n, h, r]
    hlT_all = np.ascontiguousarray(
        np.concatenate([hlw[n].T for n in range(3)], axis=1))  # [H, 3*DP]
    hr_all = np.ascontiguousarray(
        np.concatenate([hrw[n].T for n in range(3)], axis=1))
    woT = [np.ascontiguousarray(np.asarray(inputs[f"out_w{n}"],
                                           np.float32).T)
           for n in range(3)]
    bias_row = np.concatenate(
        [np.asarray(inputs[f"out_b{n}"], np.float32) for n in range(3)]
    ).reshape(1, HTOT)

    shared = {
        "wT": wT,
        "bias_col": np.ascontiguousarray(bias_col, np.float32),
        "hlT_all": hlT_all.astype(np.float32),
        "hr_all": hr_all.astype(np.float32),
        "woT0": woT[0], "woT1": woT[1], "woT2": woT[2],
        "bias_row": np.ascontiguousarray(bias_row, np.float32),
    }
    maps = []
    for i in range(B):
        mi = mask[i].astype(np.float32)                  # [S]
        L = max(mi.sum(), 1.0)
        recip = np.full((H, 1), 1.0 / L, np.float32)
        mask_bc = np.ascontiguousarray(
            np.broadcast_to(mi[None, :], (128, S))).astype(ml_dtypes.bfloat16)
        maps.append(dict(shared, x=np.ascontiguousarray(xb[i]),
                         mask_bc=mask_bc, recip_col=recip))
    return maps


_nc_cache = {}


def get_program():
    if "nc" not in _nc_cache:
        _nc_cache["nc"] = build_program()
    return _nc_cache["nc"]


def kernel(**inputs) -> np.ndarray:
    from concourse.bass_utils import run_bass_kernel_spmd

    nc = get_program()
    in_maps = make_in_maps(inputs)
    res = run_bass_kernel_spmd(nc, in_maps, list(range(N_CORES)))
    return np.concatenate([res.results[i]["out"] for i in range(B)],
                          axis=0).astype(np.float32)


# revision 7
# speedup vs baseline: 1.8622x; 1.8622x over previous
"""Trainium2 Bass kernel v2 for nn_MultiHeadCovProbeV2.

Data-parallel over batch B=8: core i processes batch i.

Key differences vs v1 baseline:
  - x is cast to bf16 on host: halves HBM traffic (32MB -> 16MB/core).
  - x is ingested PRE-TRANSPOSED via the DMA xbar transpose engine
    (dma_start(transpose=True)), so the PE does no x-transposes and the
    DVE does no PSUM->SBUF x copies. PE only does the projection
    (bf16, 1 cyc/row), back-transposes of the small [128,S] projection,
    cov accumulation, and the Newton-Schulz tail.
  - bias-add + mask-mul fused into one DVE scalar_tensor_tensor.
  - weight/head tensors are pre-laid-out on host (transposed, stacked).
"""
import sys

for p in ("/opt/trn_rl_repo", "/root/.axon_site/_ro/trn_rl_repo"):
    if p not in sys.path:
        sys.path.append(p)

import numpy as np
import ml_dtypes
import concourse.bass as bass
import concourse.mybir as mybir
from concourse.tile import TileContext
from concourse.masks import make_identity

F32 = mybir.dt.float32
BF16 = mybir.dt.bfloat16
ALU = mybir.AluOpType

B = 8
S = 2048
D = 4096
H = 64          # d_hidden
DP = 128        # d_probe
HEADS = (10, 100, 1)
HTOT = sum(HEADS)
EPS = 1e-3
NITER = 3
N_CORES = 8

SB = 512                 # s-block width (proj moving dim / pipeline grain)
NSB = S // SB            # 8
TPB = SB // 128          # s-tiles per block = 2
DCH = D // 128           # 32 d chunks
NT = S // 128            # 16


def _split_multi_waits(nc):
    """This toolchain's walrus encodes at most one sem-wait per instruction —
    and none at all on InstDmaTransposeAnt (xbar transpose: hardware encodes
    only the completion sem_num; attached waits mis-encode and hang/crash the
    queue). Move excess waits onto single-wait NOPs preceding the
    instruction on the same engine sequencer, which preserves ordering
    (the sequencer blocks on the NOP before issuing the DMA trigger)."""
    n = 0
    for f in nc.m.functions:
        for bb in f.blocks:
            out = []
            changed = False
            for inst in bb.instructions:
                si = inst.sync_info
                limit = 0 if isinstance(inst, mybir.InstDmaTransposeAnt) \
                    else 1
                if si is not None and si.on_wait and len(si.on_wait) > limit:
                    waits = list(si.on_wait)
                    keep = waits[-limit:] if limit else []
                    move = waits[:-limit] if limit else waits
                    for w in move:
                        n += 1
                        nop = mybir.InstNoOp(name=f"I-wsplit-{n}",
                                             engine=inst.engine)
                        nop.sync_info = mybir.SyncInfo(on_wait=[w],
                                                       on_update=[])
                        out.append(nop)
                    si.on_wait = keep
                    changed = True
                out.append(inst)
            if changed:
                bb.instructions = out
    return n


def build_program(split=True, reps=1, stage=99, use_act_q=False,
                  skip=(), debug_out=None):
    nc = bass.Bass()
    x = nc.dram_tensor("x", [S, D], BF16, kind="ExternalInput")
    # host-prearranged: wT[p, k*128+m] = W[m, k*128+p] (chunk-transposed)
    wT = nc.dram_tensor("wT", [128, DCH * 128], BF16, kind="ExternalInput")
    bias_col_d = nc.dram_tensor("bias_col", [2 * H, 1], F32,
                                kind="ExternalInput")
    mask_bc_d = nc.dram_tensor("mask_bc", [128, S], BF16,
                               kind="ExternalInput")
    recip_d = nc.dram_tensor("recip_col", [H, 1], F32, kind="ExternalInput")
    hlT_d = nc.dram_tensor("hlT_all", [H, 3 * DP], F32, kind="ExternalInput")
    hr_d = nc.dram_tensor("hr_all", [H, 3 * DP], F32, kind="ExternalInput")
    woT_d = [nc.dram_tensor(f"woT{n}", [DP, hs], F32, kind="ExternalInput")
             for n, hs in enumerate(HEADS)]
    brow_d = nc.dram_tensor("bias_row", [1, HTOT], F32, kind="ExternalInput")
    out = nc.dram_tensor("out", [1, HTOT], F32, kind="ExternalOutput")
    dbg = nc.dram_tensor("dbg", [H, H], F32, kind="ExternalOutput") \
        if debug_out else None

    with TileContext(nc) as tc, \
         tc.tile_pool(name="const", bufs=1) as const, \
         tc.tile_pool(name="xT", bufs=NSB) as xT_pool, \
         tc.tile_pool(name="lr", bufs=2) as lr_pool, \
         tc.tile_pool(name="lrT", bufs=2) as lrT_pool, \
         tc.tile_pool(name="ns", bufs=1) as ns_pool, \
         tc.tile_pool(name="pj", bufs=2, space="PSUM") as pj_ps, \
         tc.tile_pool(name="bt", bufs=2, space="PSUM") as bt_ps, \
         tc.tile_pool(name="sm", bufs=2, space="PSUM") as sm_ps, \
         tc.tile_pool(name="cv", bufs=2, space="PSUM") as cov_ps:

        # ---- constants ----
        ident_f = const.tile([128, 128], F32)
        ident_b = const.tile([128, 128], BF16)
        ones_col = const.tile([128, 1], F32)
        ones_row = const.tile([1, 128], F32)
        i15 = const.tile([H, H], F32)
        epsI = const.tile([H, H], F32)
        if "ident" not in skip:
            make_identity(nc, ident_f[:, :])
            nc.vector.tensor_copy(ident_b[:, :], ident_f[:, :])
            nc.vector.memset(ones_col[:, :], 1.0)
            nc.vector.memset(ones_row[:, :], 1.0)
            nc.vector.tensor_scalar_mul(i15[:, :], ident_f[0:H, 0:H], 1.5)
            nc.vector.tensor_scalar_mul(epsI[:, :], ident_f[0:H, 0:H], EPS)
        if "act" not in skip:
            act_warm = const.tile([1, 1], F32)
            nc.scalar.activation(act_warm[:, :], ones_col[0:1, 0:1],
                                 mybir.ActivationFunctionType.Sqrt)

        # ---- small input loads (host-prepped layouts) ----
        wTs = const.tile([128, DCH * 128], BF16)      # chunk k at cols k*128
        if "wts" not in skip:
            nc.sync.dma_start(out=wTs[:, :], in_=wT[:, :])
        bias_col = const.tile([128, 1], F32)
        mask_bc = const.tile([128, S], BF16)
        recip_col = const.tile([H, 1], F32)
        if "mask" not in skip:
            nc.sync.dma_start(out=bias_col[:, :], in_=bias_col_d[:, :])
            nc.sync.dma_start(out=mask_bc[:, :], in_=mask_bc_d[:, :])
            nc.sync.dma_start(out=recip_col[:, :], in_=recip_d[:, :])
        hlT_all = const.tile([H, 3 * DP], F32)
        hr_all = const.tile([H, 3 * DP], F32)
        woT = []
        for n, hs in enumerate(HEADS):
            woT.append(const.tile([DP, hs], F32, name=f"woT{n}",
                                  tag=f"woT{n}"))
        if "heads" not in skip:
            nc.sync.dma_start(out=hlT_all[:, :], in_=hlT_d[:, :])
            nc.sync.dma_start(out=hr_all[:, :], in_=hr_d[:, :])
            for n in range(3):
                nc.sync.dma_start(out=woT[n][:, :], in_=woT_d[n][:, :])
        bias_row = const.tile([1, HTOT], F32)
        nc.sync.dma_start(out=bias_row[:, :], in_=brow_d[:, :])

        def bcast_col(name, val_ap, p=H):
            ps = sm_ps.tile([128, 384], F32, name=f"pb_{name}", tag="sm")
            nc.tensor.matmul(ps[0:p, 0:1], ones_row[0:1, 0:p], val_ap,
                             start=True, stop=True)
            col = const.tile([p, 1], F32, name=f"bc_{name}", tag=f"bc_{name}")
            nc.vector.tensor_copy(col[:, :], ps[0:p, 0:1])
            return col

        # ---- main pipeline ----
        for rep in range(reps):
            rs = f"r{rep}_" if reps > 1 else ""

            if stage < 1:
                out_sb0 = ns_pool.tile([1, HTOT], F32, name=f"{rs}o0",
                                       tag="osb")
                nc.vector.tensor_copy(out_sb0[:, :], bias_row[:, :])
                nc.sync.dma_start(out=out[:, :], in_=out_sb0[:, :])
                continue

            cov = cov_ps.tile([H, H], F32, name=f"{rs}cov", tag="cv")

            for blk in range(NSB):
                xTb = xT_pool.tile([128, DCH * SB], BF16,
                                   name=f"{rs}xT{blk}", tag="xT")
                if stage == -1:
                    # natural-ingest bandwidth probe: plain contiguous loads
                    nc.sync.dma_start(
                        out=xTb[:, :].rearrange("p (t c) -> t p c", t=TPB),
                        in_=x[blk * SB:(blk + 1) * SB, :].rearrange(
                            "(t p) c -> t p c", p=128))
                    continue
                # one xbar DMA per s-block: 3D dest [128, k, s], source rows
                # are full 8KB x rows (M2S >=4KB concat keeps DMA efficient)
                q = nc.sync if (not use_act_q or blk % 2 == 0) else nc.scalar
                q.dma_start(
                    out=xTb[:, :].rearrange("p (k s) -> p k s", k=DCH),
                    in_=x[blk * SB:(blk + 1) * SB, :],
                    transpose=True)
                if stage < 2:
                    continue

                pp = pj_ps.tile([128, SB], F32, name=f"{rs}pp{blk}", tag="pj")
                for k in range(DCH):
                    nc.tensor.matmul(pp[:, :], wTs[:, k * 128:(k + 1) * 128],
                                     xTb[:, k * SB:(k + 1) * SB],
                                     start=(k == 0), stop=(k == DCH - 1))
                # lr = (pp + bias) * mask  (bias per-partition, mask per-col)
                lr = lr_pool.tile([128, SB], BF16, name=f"{rs}lr{blk}",
                                  tag="lr")
                nc.vector.scalar_tensor_tensor(
                    lr[:, :], pp[:, :], bias_col[:, 0:1],
                    mask_bc[:, blk * SB:(blk + 1) * SB],
                    op0=ALU.add, op1=ALU.mult)
                if stage < 3:
                    continue

                btp = bt_ps.tile([128, SB], BF16, name=f"{rs}bt{blk}",
                                 tag="bt")
                for j in range(TPB):
                    nc.tensor.transpose(btp[:, j * 128:(j + 1) * 128],
                                        lr[:, j * 128:(j + 1) * 128],
                                        ident_b[:, :])
                lrT = lrT_pool.tile([128, SB], BF16, name=f"{rs}lrT{blk}",
                                    tag="lrT")
                nc.vector.tensor_copy(lrT[:, :], btp[:, :])
                for j in range(TPB):
                    t = blk * TPB + j
                    nc.tensor.matmul(cov[:, :],
                                     lrT[:, j * 128:j * 128 + H],
                                     lrT[:, j * 128 + H:(j + 1) * 128],
                                     start=(t == 0), stop=(t == NT - 1))

            if stage < 4:
                out_sb0 = ns_pool.tile([1, HTOT], F32, name=f"{rs}o0",
                                       tag="osb")
                if stage == 3:
                    nc.vector.tensor_copy(out_sb0[0:1, 0:H],
                                          cov[0:1, 0:H])
                    nc.vector.tensor_copy(out_sb0[0:1, H:HTOT],
                                          bias_row[0:1, H:HTOT])
                else:
                    nc.vector.tensor_copy(out_sb0[:, :], bias_row[:, :])
                nc.sync.dma_start(out=out[:, :], in_=out_sb0[:, :])
                continue

            # ---- A = cov*recipL + eps*I ----
            A = ns_pool.tile([H, H], F32, name=f"{rs}A", tag="A")
            nc.vector.scalar_tensor_tensor(A[:, :], cov[:, :],
                                           recip_col[:, 0:1], epsI[:, :],
                                           op0=ALU.mult, op1=ALU.add)

            if debug_out == "A":
                nc.sync.dma_start(out=dbg[:, :], in_=A[:, :])

            # ---- Frobenius norm ----
            scr = ns_pool.tile([H, H], F32, name=f"{rs}scr", tag="scr")
            sq_col = ns_pool.tile([H, 1], F32, name=f"{rs}sq", tag="sq")
            nc.vector.tensor_tensor(out=scr[:, :], in0=A[:, :], in1=A[:, :],
                                    op=ALU.mult)
            nc.vector.reduce_sum(sq_col[:, :], scr[:, :],
                                 axis=mybir.AxisListType.X)
            pS = sm_ps.tile([1, 384], F32, name=f"{rs}pS", tag="sm")
            nc.tensor.matmul(pS[0:1, 0:1], sq_col[:, :], ones_col[0:H, :],
                             start=True, stop=True)
            normA = ns_pool.tile([1, 1], F32, name=f"{rs}normA", tag="normA")
            nc.scalar.activation(normA[:, :], pS[0:1, 0:1],
                                 mybir.ActivationFunctionType.Sqrt)
            snorm = ns_pool.tile([1, 1], F32, name=f"{rs}sn", tag="sn")
            nc.scalar.activation(snorm[:, :], normA[:, :],
                                 mybir.ActivationFunctionType.Sqrt)
            rnorm = ns_pool.tile([1, 1], F32, name=f"{rs}rn", tag="rn")
            nc.vector.reciprocal(rnorm[:, :], normA[:, :])
            rnorm_h = bcast_col(f"{rs}rnorm", rnorm[0:1, 0:1])
            snorm_h = bcast_col(f"{rs}snorm", snorm[0:1, 0:1])

            # ---- Newton-Schulz, transpose-tracked, T1 = 1.5I - 0.5*Y0 ----
            def mm(name, lhsT, rhs, m=H, n=H):
                ps = sm_ps.tile([128, 384], F32, name=f"{rs}ps_{name}",
                                tag="sm")
                nc.tensor.matmul(ps[0:m, 0:n], lhsT, rhs, start=True,
                                 stop=True)
                return ps[0:m, 0:n]

            def to_sb(name, ps_ap, m=H, n=H):
                sb = ns_pool.tile([m, n], F32, name=f"{rs}{name}", tag=name)
                nc.vector.tensor_copy(sb[:, :], ps_ap)
                return sb

            def half_i15(name, src_ap):
                t = ns_pool.tile([H, H], F32, name=f"{rs}{name}", tag=name)
                nc.vector.scalar_tensor_tensor(t[:, :], src_ap, -0.5,
                                               i15[:, :], op0=ALU.mult,
                                               op1=ALU.add)
                return t

            Y0 = ns_pool.tile([H, H], F32, name=f"{rs}Y0", tag="Y0")
            nc.vector.tensor_scalar_mul(Y0[:, :], A[:, :], rnorm_h[:, 0:1])
            Yt0 = to_sb("Yt0", mm("yt0", Y0[:, :], ident_f[0:H, 0:H]))
            T1 = half_i15("T1", Y0[:, :])
            T1t = half_i15("T1t", Yt0[:, :])
            Y1 = to_sb("Y1", mm("y1", Yt0[:, :], T1[:, :]))
            Yt1 = to_sb("Yt1", mm("yt1", T1[:, :], Yt0[:, :]))
            W2 = mm("w2", T1t[:, :], Y1[:, :])
            W2t = mm("w2t", Y1[:, :], T1t[:, :])
            T2 = half_i15("T2", W2)
            T2t = half_i15("T2t", W2t)
            Y2 = to_sb("Y2", mm("y2", Yt1[:, :], T2[:, :]))
            Yt2 = to_sb("Yt2", mm("yt2", T2[:, :], Yt1[:, :]))
            Z2t = to_sb("Z2t", mm("z2t", T1[:, :], T2t[:, :]))
            W3 = mm("w3", Z2t[:, :], Y2[:, :])
            T3 = half_i15("T3", W3)
            Y3 = mm("y3", Yt2[:, :], T3[:, :])
            Ys = ns_pool.tile([H, H], F32, name=f"{rs}Ys", tag="Ys")
            nc.vector.tensor_scalar_mul(Ys[:, :], Y3, snorm_h[:, 0:1])
            if debug_out == "Ys":
                nc.sync.dma_start(out=dbg[:, :], in_=Ys[:, :])

            # ---- heads: hidden[n,h] = sum_{l,r} Ys[l,r] hl[n,h,l] hr[n,h,r]
            MhT = mm("mht", Ys[:, :], hlT_all[:, :], m=H, n=3 * DP)
            prod = ns_pool.tile([H, 3 * DP], F32, name=f"{rs}prod",
                                tag="prod")
            nc.vector.tensor_tensor(out=prod[:, :], in0=MhT,
                                    in1=hr_all[:, :], op=ALU.mult)
            hid_ps = sm_ps.tile([128, 384], F32, name=f"{rs}hid", tag="sm")
            for n in range(3):
                nc.tensor.matmul(hid_ps[:, n:n + 1],
                                 prod[:, n * DP:(n + 1) * DP],
                                 ones_col[0:H, 0:1], start=True, stop=True)
            hid_sb = ns_pool.tile([128, 3], F32, name=f"{rs}hsb", tag="hsb")
            nc.vector.tensor_copy(hid_sb[:, :], hid_ps[:, 0:3])
            out_ps = sm_ps.tile([1, 384], F32, name=f"{rs}ops", tag="sm")
            off = 0
            for n, hs in enumerate(HEADS):
                nc.tensor.matmul(out_ps[0:1, off:off + hs],
                                 hid_sb[:, n:n + 1], woT[n][:, :],
                                 start=True, stop=True)
                off += hs
            out_sb = ns_pool.tile([1, HTOT], F32, name=f"{rs}osb", tag="osb")
            nc.vector.tensor_tensor(out=out_sb[:, :], in0=out_ps[0:1, 0:HTOT],
                                    in1=bias_row[:, :], op=ALU.add)
            nc.sync.dma_start(out=out[:, :], in_=out_sb[:, :])

    if split:
        _split_multi_waits(nc)
    return nc


def make_in_maps(inputs):
    x = np.asarray(inputs["x"])
    mask = np.asarray(inputs["attn_mask"])
    xb = x.astype(ml_dtypes.bfloat16)

    wl = np.asarray(inputs["proj_left_w"], np.float32)
    wr = np.asarray(inputs["proj_right_w"], np.float32)
    w_st = np.concatenate([wl, wr], axis=0)              # [128, D]
    # wT[p, c*128+m] = W[m, c*128+p]: per-chunk transpose, laid out flat
    wT = np.ascontiguousarray(
        w_st.reshape(128, D // 128, 128).transpose(2, 1, 0).reshape(128, D)
    ).astype(ml_dtypes.bfloat16)
    bias_col = np.concatenate(
        [np.asarray(inputs["proj_left_b"], np.float32),
         np.asarray(inputs["proj_right_b"], np.float32)]).reshape(2 * H, 1)

    hlw = np.asarray(inputs["head_left"], np.float32)    # [3, DP, H]
    hrw = np.asarray(inputs["head_right"], np.float32)
    # hlT_all[l, n*DP+h] = hl[n, h, l];  hr_all[r, n*DP+h] = hr[n, h, r]
    hlT_all = np.ascontiguousarray(
        np.concatenate([hlw[n].T for n in range(3)], axis=1))  # [H, 3*DP]
    hr_all = np.ascontiguousarray(
        np.concatenate([hrw[n].T for n in range(3)], axis=1))
    woT = [np.ascontiguousarray(np.asarray(inputs[f"out_w{n}"],
                                           np.float32).T)
           for n in range(3)]
    bias_row = np.concatenate(
        [np.asarray(inputs[f"out_b{n}"], np.float32) for n in range(3)]
    ).reshape(1, HTOT)

    shared = {
        "wT": wT,
        "bias_col": np.ascontiguousarray(bias_col, np.float32),
        "hlT_all": hlT_all.astype(np.float32),
        "hr_all": hr_all.astype(np.float32),
        "woT0": woT[0], "woT1": woT[1], "woT2": woT[2],
        "bias_row": np.ascontiguousarray(bias_row, np.float32),
    }
    maps = []
    for i in range(B):
        mi = mask[i].astype(np.float32)                  # [S]
        L = max(mi.sum(), 1.0)
        recip = np.full((H, 1), 1.0 / L, np.float32)
        mask_bc = np.ascontiguousarray(
            np.broadcast_to(mi[None, :], (128, S))).astype(ml_dtypes.bfloat16)
        maps.append(dict(shared, x=np.ascontiguousarray(xb[i]),
                         mask_bc=mask_bc, recip_col=recip))
    return maps


_nc_cache = {}


def get_program():
    if "nc" not in _nc_cache:
        _nc_cache["nc"] = build_program()
    return _nc_cache["nc"]


def kernel(**inputs) -> np.ndarray:
    from concourse.bass_utils import run_bass_kernel_spmd

    nc = get_program()
    in_maps = make_in_maps(inputs)
    res = run_bass_kernel_spmd(nc, in_maps, list(range(N_CORES)))
    return np.concatenate([res.results[i]["out"] for i in range(B)],
                          axis=0).astype(np.float32)


# revision 9
# speedup vs baseline: 2.0251x; 1.0875x over previous
"""Trainium2 Bass kernel v2 for nn_MultiHeadCovProbeV2.

Data-parallel over batch B=8: core i processes batch i.

Key differences vs v1 baseline:
  - x is cast to bf16 on host: halves HBM traffic (32MB -> 16MB/core).
  - x is ingested PRE-TRANSPOSED via the DMA xbar transpose engine
    (dma_start(transpose=True)), so the PE does no x-transposes and the
    DVE does no PSUM->SBUF x copies. PE only does the projection
    (bf16, 1 cyc/row), back-transposes of the small [128,S] projection,
    cov accumulation, and the Newton-Schulz tail.
  - bias-add + mask-mul fused into one DVE scalar_tensor_tensor.
  - weight/head tensors are pre-laid-out on host (transposed, stacked).
"""
import sys

for p in ("/opt/trn_rl_repo", "/root/.axon_site/_ro/trn_rl_repo"):
    if p not in sys.path:
        sys.path.append(p)

import numpy as np
import ml_dtypes
import concourse.bass as bass
import concourse.mybir as mybir
from concourse.tile import TileContext
from concourse.masks import make_identity

F32 = mybir.dt.float32
BF16 = mybir.dt.bfloat16
ALU = mybir.AluOpType

B = 8
S = 2048
D = 4096
H = 64          # d_hidden
DP = 128        # d_probe
HEADS = (10, 100, 1)
HTOT = sum(HEADS)
EPS = 1e-3
NITER = 3
N_CORES = 8

SB = 512                 # s-block width (proj moving dim / pipeline grain)
NSB = S // SB            # 8
TPB = SB // 128          # s-tiles per block = 2
DCH = D // 128           # 32 d chunks
NT = S // 128            # 16


def _split_multi_waits(nc):
    """This toolchain's walrus encodes at most one sem-wait per instruction —
    and none at all on InstDmaTransposeAnt (xbar transpose: hardware encodes
    only the completion sem_num; attached waits mis-encode and hang/crash the
    queue). Move excess waits onto single-wait NOPs preceding the
    instruction on the same engine sequencer, which preserves ordering
    (the sequencer blocks on the NOP before issuing the DMA trigger)."""
    n = 0
    for f in nc.m.functions:
        for bb in f.blocks:
            out = []
            changed = False
            for inst in bb.instructions:
                si = inst.sync_info
                limit = 0 if isinstance(inst, mybir.InstDmaTransposeAnt) \
                    else 1
                if si is not None and si.on_wait and len(si.on_wait) > limit:
                    waits = list(si.on_wait)
                    keep = waits[-limit:] if limit else []
                    move = waits[:-limit] if limit else waits
                    for w in move:
                        n += 1
                        nop = mybir.InstNoOp(name=f"I-wsplit-{n}",
                                             engine=inst.engine)
                        nop.sync_info = mybir.SyncInfo(on_wait=[w],
                                                       on_update=[])
                        out.append(nop)
                    si.on_wait = keep
                    changed = True
                out.append(inst)
            if changed:
                bb.instructions = out
    return n


def build_program(split=True, reps=1, stage=99, use_act_q=False,
                  skip=(), debug_out=None):
    nc = bass.Bass()
    x = nc.dram_tensor("x", [S, D], BF16, kind="ExternalInput")
    # host-prearranged: wT[p, k*128+m] = W[m, k*128+p] (chunk-transposed)
    wT = nc.dram_tensor("wT", [128, DCH * 128], BF16, kind="ExternalInput")
    bias_col_d = nc.dram_tensor("bias_col", [2 * H, 1], F32,
                                kind="ExternalInput")
    mask_bc_d = nc.dram_tensor("mask_bc", [128, S], BF16,
                               kind="ExternalInput")
    recip_d = nc.dram_tensor("recip_col", [H, 1], F32, kind="ExternalInput")
    hlT_d = nc.dram_tensor("hlT_all", [H, 3 * DP], F32, kind="ExternalInput")
    hr_d = nc.dram_tensor("hr_all", [H, 3 * DP], F32, kind="ExternalInput")
    woT_d = [nc.dram_tensor(f"woT{n}", [DP, hs], F32, kind="ExternalInput")
             for n, hs in enumerate(HEADS)]
    brow_d = nc.dram_tensor("bias_row", [1, HTOT], F32, kind="ExternalInput")
    out = nc.dram_tensor("out", [1, HTOT], F32, kind="ExternalOutput")
    dbg = nc.dram_tensor("dbg", [H, H], F32, kind="ExternalOutput") \
        if debug_out else None

    with TileContext(nc) as tc, \
         tc.tile_pool(name="const", bufs=1) as const, \
         tc.tile_pool(name="xT", bufs=NSB) as xT_pool, \
         tc.tile_pool(name="lr", bufs=2) as lr_pool, \
         tc.tile_pool(name="lrT", bufs=2) as lrT_pool, \
         tc.tile_pool(name="ns", bufs=1) as ns_pool, \
         tc.tile_pool(name="pj", bufs=2, space="PSUM") as pj_ps, \
         tc.tile_pool(name="bt", bufs=2, space="PSUM") as bt_ps, \
         tc.tile_pool(name="sm", bufs=2, space="PSUM") as sm_ps, \
         tc.tile_pool(name="cv", bufs=2, space="PSUM") as cov_ps:

        # ---- constants ----
        ident_f = const.tile([128, 128], F32)
        ident_b = const.tile([128, 128], BF16)
        ones_col = const.tile([128, 1], F32)
        ones_row = const.tile([1, 128], F32)
        i15 = const.tile([H, H], F32)
        epsI = const.tile([H, H], F32)
        ones_hh = const.tile([H, H], F32)
        if "ident" not in skip:
            make_identity(nc, ident_f[:, :])
            nc.vector.tensor_copy(ident_b[:, :], ident_f[:, :])
            nc.vector.memset(ones_col[:, :], 1.0)
            nc.vector.memset(ones_row[:, :], 1.0)
            nc.vector.memset(ones_hh[:, :], 1.0)
            nc.vector.tensor_scalar_mul(i15[:, :], ident_f[0:H, 0:H], 1.5)
            nc.vector.tensor_scalar_mul(epsI[:, :], ident_f[0:H, 0:H], EPS)
        if "act" not in skip:
            act_warm = const.tile([1, 1], F32)
            nc.scalar.activation(act_warm[:, :], ones_col[0:1, 0:1],
                                 mybir.ActivationFunctionType.Sqrt)

        # ---- small input loads (host-prepped layouts) ----
        wTs = const.tile([128, DCH * 128], BF16)      # chunk k at cols k*128
        if "wts" not in skip:
            nc.sync.dma_start(out=wTs[:, :], in_=wT[:, :])
        bias_col = const.tile([128, 1], F32)
        mask_bc = const.tile([128, S], BF16)
        recip_col = const.tile([H, 1], F32)
        if "mask" not in skip:
            nc.sync.dma_start(out=bias_col[:, :], in_=bias_col_d[:, :])
            nc.sync.dma_start(out=mask_bc[:, :], in_=mask_bc_d[:, :])
            nc.sync.dma_start(out=recip_col[:, :], in_=recip_d[:, :])
        hlT_all = const.tile([H, 3 * DP], F32)
        hr_all = const.tile([H, 3 * DP], F32)
        woT = []
        for n, hs in enumerate(HEADS):
            woT.append(const.tile([DP, hs], F32, name=f"woT{n}",
                                  tag=f"woT{n}"))
        if "heads" not in skip:
            nc.sync.dma_start(out=hlT_all[:, :], in_=hlT_d[:, :])
            nc.sync.dma_start(out=hr_all[:, :], in_=hr_d[:, :])
            for n in range(3):
                nc.sync.dma_start(out=woT[n][:, :], in_=woT_d[n][:, :])
        bias_row = const.tile([1, HTOT], F32)
        nc.sync.dma_start(out=bias_row[:, :], in_=brow_d[:, :])

        def bcast_col(name, val_ap, p=H):
            ps = sm_ps.tile([128, 384], F32, name=f"pb_{name}", tag="sm")
            nc.tensor.matmul(ps[0:p, 0:1], ones_row[0:1, 0:p], val_ap,
                             start=True, stop=True)
            col = const.tile([p, 1], F32, name=f"bc_{name}", tag=f"bc_{name}")
            nc.vector.tensor_copy(col[:, :], ps[0:p, 0:1])
            return col

        # ---- main pipeline ----
        for rep in range(reps):
            rs = f"r{rep}_" if reps > 1 else ""

            if stage < 1:
                out_sb0 = ns_pool.tile([1, HTOT], F32, name=f"{rs}o0",
                                       tag="osb")
                nc.vector.tensor_copy(out_sb0[:, :], bias_row[:, :])
                nc.sync.dma_start(out=out[:, :], in_=out_sb0[:, :])
                continue

            cov = cov_ps.tile([H, H], F32, name=f"{rs}cov", tag="cv")

            for blk in range(NSB):
                xTb = xT_pool.tile([128, DCH * SB], BF16,
                                   name=f"{rs}xT{blk}", tag="xT")
                if stage == -1:
                    # natural-ingest bandwidth probe: plain contiguous loads
                    nc.sync.dma_start(
                        out=xTb[:, :].rearrange("p (t c) -> t p c", t=TPB),
                        in_=x[blk * SB:(blk + 1) * SB, :].rearrange(
                            "(t p) c -> t p c", p=128))
                    continue
                # one xbar DMA per s-block: 3D dest [128, k, s], source rows
                # are full 8KB x rows (M2S >=4KB concat keeps DMA efficient)
                q = nc.sync if (not use_act_q or blk % 2 == 0) else nc.scalar
                q.dma_start(
                    out=xTb[:, :].rearrange("p (k s) -> p k s", k=DCH),
                    in_=x[blk * SB:(blk + 1) * SB, :],
                    transpose=True)
                if stage < 2:
                    continue

                pp = pj_ps.tile([128, SB], F32, name=f"{rs}pp{blk}", tag="pj")
                for k in range(DCH):
                    nc.tensor.matmul(pp[:, :], wTs[:, k * 128:(k + 1) * 128],
                                     xTb[:, k * SB:(k + 1) * SB],
                                     start=(k == 0), stop=(k == DCH - 1))
                # lr = (pp + bias) * mask  (bias per-partition, mask per-col)
                lr = lr_pool.tile([128, SB], BF16, name=f"{rs}lr{blk}",
                                  tag="lr")
                nc.vector.scalar_tensor_tensor(
                    lr[:, :], pp[:, :], bias_col[:, 0:1],
                    mask_bc[:, blk * SB:(blk + 1) * SB],
                    op0=ALU.add, op1=ALU.mult)
                if stage < 3:
                    continue

                btp = bt_ps.tile([128, SB], BF16, name=f"{rs}bt{blk}",
                                 tag="bt")
                for j in range(TPB):
                    nc.tensor.transpose(btp[:, j * 128:(j + 1) * 128],
                                        lr[:, j * 128:(j + 1) * 128],
                                        ident_b[:, :])
                lrT = lrT_pool.tile([128, SB], BF16, name=f"{rs}lrT{blk}",
                                    tag="lrT")
                nc.vector.tensor_copy(lrT[:, :], btp[:, :])
                for j in range(TPB):
                    t = blk * TPB + j
                    nc.tensor.matmul(cov[:, :],
                                     lrT[:, j * 128:j * 128 + H],
                                     lrT[:, j * 128 + H:(j + 1) * 128],
                                     start=(t == 0), stop=(t == NT - 1))

            if stage < 4:
                out_sb0 = ns_pool.tile([1, HTOT], F32, name=f"{rs}o0",
                                       tag="osb")
                if stage == 3:
                    nc.vector.tensor_copy(out_sb0[0:1, 0:H],
                                          cov[0:1, 0:H])
                    nc.vector.tensor_copy(out_sb0[0:1, H:HTOT],
                                          bias_row[0:1, H:HTOT])
                else:
                    nc.vector.tensor_copy(out_sb0[:, :], bias_row[:, :])
                nc.sync.dma_start(out=out[:, :], in_=out_sb0[:, :])
                continue

            # ---- A = cov*recipL + eps*I ----
            A = ns_pool.tile([H, H], F32, name=f"{rs}A", tag="A")
            nc.vector.scalar_tensor_tensor(A[:, :], cov[:, :],
                                           recip_col[:, 0:1], epsI[:, :],
                                           op0=ALU.mult, op1=ALU.add)

            if debug_out == "A":
                nc.sync.dma_start(out=dbg[:, :], in_=A[:, :])

            # ---- Frobenius norm ----
            scr = ns_pool.tile([H, H], F32, name=f"{rs}scr", tag="scr")
            sq_col = ns_pool.tile([H, 1], F32, name=f"{rs}sq", tag="sq")
            nc.vector.tensor_tensor(out=scr[:, :], in0=A[:, :], in1=A[:, :],
                                    op=ALU.mult)
            nc.vector.reduce_sum(sq_col[:, :], scr[:, :],
                                 axis=mybir.AxisListType.X)
            # ones[HxH] @ sq_col replicates the total to all H partitions:
            # sqrt/reciprocal then yield ready-to-use [H,1] scale columns
            pS = sm_ps.tile([128, 384], F32, name=f"{rs}pS", tag="sm")
            nc.tensor.matmul(pS[0:H, 0:1], ones_hh[:, :], sq_col[:, :],
                             start=True, stop=True)
            normA = ns_pool.tile([H, 1], F32, name=f"{rs}normA", tag="normA")
            nc.scalar.activation(normA[:, :], pS[0:H, 0:1],
                                 mybir.ActivationFunctionType.Sqrt)
            snorm_h = ns_pool.tile([H, 1], F32, name=f"{rs}sn", tag="sn")
            nc.scalar.activation(snorm_h[:, :], normA[:, :],
                                 mybir.ActivationFunctionType.Sqrt)
            rnorm_h = ns_pool.tile([H, 1], F32, name=f"{rs}rn", tag="rn")
            nc.vector.reciprocal(rnorm_h[:, :], normA[:, :])

            # ---- Newton-Schulz, transpose-tracked, T1 = 1.5I - 0.5*Y0 ----
            def mm(name, lhsT, rhs, m=H, n=H):
                ps = sm_ps.tile([128, 384], F32, name=f"{rs}ps_{name}",
                                tag="sm")
                nc.tensor.matmul(ps[0:m, 0:n], lhsT, rhs, start=True,
                                 stop=True)
                return ps[0:m, 0:n]

            def to_sb(name, ps_ap, m=H, n=H):
                sb = ns_pool.tile([m, n], F32, name=f"{rs}{name}", tag=name)
                nc.vector.tensor_copy(sb[:, :], ps_ap)
                return sb

            def half_i15(name, src_ap):
                t = ns_pool.tile([H, H], F32, name=f"{rs}{name}", tag=name)
                nc.vector.scalar_tensor_tensor(t[:, :], src_ap, -0.5,
                                               i15[:, :], op0=ALU.mult,
                                               op1=ALU.add)
                return t

            Y0 = ns_pool.tile([H, H], F32, name=f"{rs}Y0", tag="Y0")
            nc.vector.tensor_scalar_mul(Y0[:, :], A[:, :], rnorm_h[:, 0:1])
            Yt0 = to_sb("Yt0", mm("yt0", Y0[:, :], ident_f[0:H, 0:H]))
            T1 = half_i15("T1", Y0[:, :])
            T1t = half_i15("T1t", Yt0[:, :])
            Y1 = to_sb("Y1", mm("y1", Yt0[:, :], T1[:, :]))
            Yt1 = to_sb("Yt1", mm("yt1", T1[:, :], Yt0[:, :]))
            W2 = mm("w2", T1t[:, :], Y1[:, :])
            W2t = mm("w2t", Y1[:, :], T1t[:, :])
            T2 = half_i15("T2", W2)
            T2t = half_i15("T2t", W2t)
            Y2 = to_sb("Y2", mm("y2", Yt1[:, :], T2[:, :]))
            Yt2 = to_sb("Yt2", mm("yt2", T2[:, :], Yt1[:, :]))
            Z2t = to_sb("Z2t", mm("z2t", T1[:, :], T2t[:, :]))
            W3 = mm("w3", Z2t[:, :], Y2[:, :])
            T3 = half_i15("T3", W3)
            Y3 = mm("y3", Yt2[:, :], T3[:, :])
            Ys = ns_pool.tile([H, H], F32, name=f"{rs}Ys", tag="Ys")
            nc.vector.tensor_scalar_mul(Ys[:, :], Y3, snorm_h[:, 0:1])
            if debug_out == "Ys":
                nc.sync.dma_start(out=dbg[:, :], in_=Ys[:, :])

            # ---- heads: hidden[n,h] = sum_{l,r} Ys[l,r] hl[n,h,l] hr[n,h,r]
            MhT = mm("mht", Ys[:, :], hlT_all[:, :], m=H, n=3 * DP)
            prod = ns_pool.tile([H, 3 * DP], F32, name=f"{rs}prod",
                                tag="prod")
            nc.vector.tensor_tensor(out=prod[:, :], in0=MhT,
                                    in1=hr_all[:, :], op=ALU.mult)
            hid_ps = sm_ps.tile([128, 384], F32, name=f"{rs}hid", tag="sm")
            for n in range(3):
                nc.tensor.matmul(hid_ps[:, n:n + 1],
                                 prod[:, n * DP:(n + 1) * DP],
                                 ones_col[0:H, 0:1], start=True, stop=True)
            hid_sb = ns_pool.tile([128, 3], F32, name=f"{rs}hsb", tag="hsb")
            nc.vector.tensor_copy(hid_sb[:, :], hid_ps[:, 0:3])
            out_ps = sm_ps.tile([1, 384], F32, name=f"{rs}ops", tag="sm")
            off = 0
            for n, hs in enumerate(HEADS):
                nc.tensor.matmul(out_ps[0:1, off:off + hs],
                                 hid_sb[:, n:n + 1], woT[n][:, :],
                                 start=True, stop=True)
                off += hs
            out_sb = ns_pool.tile([1, HTOT], F32, name=f"{rs}osb", tag="osb")
            nc.vector.tensor_tensor(out=out_sb[:, :], in0=out_ps[0:1, 0:HTOT],
                                    in1=bias_row[:, :], op=ALU.add)
            nc.sync.dma_start(out=out[:, :], in_=out_sb[:, :])

    if split:
        _split_multi_waits(nc)
    return nc


def make_in_maps(inputs):
    x = np.asarray(inputs["x"])
    mask = np.asarray(inputs["attn_mask"])
    xb = x.astype(ml_dtypes.bfloat16)

    wl = np.asarray(inputs["proj_left_w"], np.float32)
    wr = np.asarray(inputs["proj_right_w"], np.float32)
    w_st = np.concatenate([wl, wr], axis=0)              # [128, D]
    # wT[p, c*128+m] = W[m, c*128+p]: per-chunk transpose, laid out flat
    wT = np.ascontiguousarray(
        w_st.reshape(128, D // 128, 128).transpose(2, 1, 0).reshape(128, D)
    ).astype(ml_dtypes.bfloat16)
    bias_col = np.concatenate(
        [np.asarray(inputs["proj_left_b"], np.float32),
         np.asarray(inputs["proj_right_b"], np.float32)]).reshape(2 * H, 1)

    hlw = np.asarray(inputs["head_left"], np.float32)    # [3, DP, H]
    hrw = np.asarray(inputs["head_right"], np.float32)
    # hlT_all[l, n*DP+h] = hl[n, h, l];  hr_all[r, n*DP+h] = hr[n, h, r]
    hlT_all = np.ascontiguousarray(
        np.concatenate([hlw[n].T for n in range(3)], axis=1))  # [H, 3*DP]
    hr_all = np.ascontiguousarray(
        np.concatenate([hrw[n].T for n in range(3)], axis=1))
    woT = [np.ascontiguousarray(np.asarray(inputs[f"out_w{n}"],
                                           np.float32).T)
           for n in range(3)]
    bias_row = np.concatenate(
        [np.asarray(inputs[f"out_b{n}"], np.float32) for n in range(3)]
    ).reshape(1, HTOT)

    shared = {
        "wT": wT,
        "bias_col": np.ascontiguousarray(bias_col, np.float32),
        "hlT_all": hlT_all.astype(np.float32),
        "hr_all": hr_all.astype(np.float32),
        "woT0": woT[0], "woT1": woT[1], "woT2": woT[2],
        "bias_row": np.ascontiguousarray(bias_row, np.float32),
    }
    maps = []
    for i in range(B):
        mi = mask[i].astype(np.float32)                  # [S]
        L = max(mi.sum(), 1.0)
        recip = np.full((H, 1), 1.0 / L, np.float32)
        mask_bc = np.ascontiguousarray(
            np.broadcast_to(mi[None, :], (128, S))).astype(ml_dtypes.bfloat16)
        maps.append(dict(shared, x=np.ascontiguousarray(xb[i]),
                         mask_bc=mask_bc, recip_col=recip))
    return maps


_nc_cache = {}


def get_program():
    if "nc" not in _nc_cache:
        _nc_cache["nc"] = build_program()
    return _nc_cache["nc"]


def kernel(**inputs) -> np.ndarray:
    from concourse.bass_utils import run_bass_kernel_spmd

    nc = get_program()
    in_maps = make_in_maps(inputs)
    res = run_bass_kernel_spmd(nc, in_maps, list(range(N_CORES)))
    return np.concatenate([res.results[i]["out"] for i in range(B)],
                          axis=0).astype(np.float32)
